# revision 6
# baseline (speedup 1.0000x reference)
"""Bidirectional LSTM layer (T=256, B=64, I=H=1024) on 8 Trainium2 NeuronCores.

Distribution: 8-way hidden split. Core c owns hidden units [128c, 128c+128)
of BOTH directions; its per-step gate slice is 512 cols per direction
(gate-col order [i|f|o|g] x 128, so sigmoid covers cols 0:384, tanh 384:512).

Per step t (one Bass program, fully unrolled, Tile-scheduled):
  - fp32r matmuls accumulate gates into per-direction PSUM banks:
    K=1 bias matmul + 8 input-projection (xg) matmuls (emitted D_LOOK steps
    ahead, so they execute inside the AllGather window) + 8 recurrent
    matmuls vs the allgathered h.T of step t-1.
  - sigmoid/tanh on ScalarE, c/h update on VectorE, on [128,*] tiles
    (forward batch in partitions 0:64, backward in 64:128).
  - h [128,128] is PE-transposed to h.T [hid, batch] and AllGathered
    (ncfw collective, DRAM bounce) into every core's next stationaries.
The backward direction is the same recurrence fed with time-reversed x
(pure host-side indexing); outputs are un-reversed at assembly.
"""
import numpy as np

T, B, I, H = 256, 64, 1024, 1024
N_CORES = 8
GL = 512          # local gate cols per core per direction
KC = 8            # K chunks (1024 / 128)
D_LOOK = 2        # xg lookahead steps (PSUM banks: 2*(D_LOOK+1) + 1 <= 8)

_CACHE = {}


def _build():
    import concourse.bacc as bacc
    import concourse.mybir as mybir
    import concourse.tile as tile
    from concourse.masks import make_identity

    dt = mybir.dt
    f32 = dt.float32
    f32r = dt.float32r

    nc = bacc.Bacc(None)
    xt = nc.declare_dram_parameter("xt", [T, 128, KC * B], f32, isOutput=False)
    whh_f = nc.declare_dram_parameter("whh_f", [H, GL], f32, isOutput=False)
    whh_b = nc.declare_dram_parameter("whh_b", [H, GL], f32, isOutput=False)
    wih_f = nc.declare_dram_parameter("wih_f", [I, GL], f32, isOutput=False)
    wih_b = nc.declare_dram_parameter("wih_b", [I, GL], f32, isOutput=False)
    bias_f = nc.declare_dram_parameter("bias_f", [1, GL], f32, isOutput=False)
    bias_b = nc.declare_dram_parameter("bias_b", [1, GL], f32, isOutput=False)
    ones_in = nc.declare_dram_parameter("ones", [1, B], f32, isOutput=False)
    out_slab = nc.declare_dram_parameter("out_slab", [T, 128, 128], f32, isOutput=True)
    hc_out = nc.declare_dram_parameter("hc_out", [2, 128, 128], f32, isOutput=True)

    with tile.TileContext(nc) as tc:
        with (
            tc.tile_pool(name="const", bufs=1) as const,
            tc.tile_pool(name="sb", bufs=2) as sb,
            tc.tile_pool(name="state", bufs=2) as state,
            tc.tile_pool(name="pgates", bufs=2 * (D_LOOK + 1), space="PSUM") as pgates,
            tc.tile_pool(name="ptr", bufs=1, space="PSUM") as ptr,
            tc.tile_pool(name="pwarm", bufs=1, space="PSUM") as pwarm,
            tc.tile_pool(name="dram", bufs=2, space="DRAM") as dram,
        ):
            whhf_t = const.tile([128, KC, GL], f32r, tag="whhf")
            whhb_t = const.tile([128, KC, GL], f32r, tag="whhb")
            wihf_t = const.tile([128, KC, GL], f32r, tag="wihf")
            wihb_t = const.tile([128, KC, GL], f32r, tag="wihb")
            for tle, src in ((whhf_t, whh_f), (whhb_t, whh_b),
                             (wihf_t, wih_f), (wihb_t, wih_b)):
                nc.gpsimd.dma_start(tle[:], src.rearrange("(c p) n -> p c n", p=128))
            bf_t = const.tile([1, GL], f32r, tag="bf")
            bb_t = const.tile([1, GL], f32r, tag="bb")
            ones_t = const.tile([1, B], f32r, tag="ones")
            nc.gpsimd.dma_start(bf_t[:], bias_f[:])
            nc.gpsimd.dma_start(bb_t[:], bias_b[:])
            nc.gpsimd.dma_start(ones_t[:], ones_in[:])
            ident = const.tile([128, 128], f32, tag="ident")
            make_identity(nc, ident)
            c_zero = const.tile([128, 128], f32, tag="czero")
            nc.vector.memset(c_zero[:], 0.0)

            banks = {}

            def emit_xg(s):
                bank_f = pgates.tile([64, GL], f32, tag="gates")
                bank_b = pgates.tile([64, GL], f32, tag="gates")
                banks[s] = (bank_f, bank_b)
                xf32 = sb.tile([128, KC * B], f32, tag="xf32")
                xb32 = sb.tile([128, KC * B], f32, tag="xb32")
                nc.sync.dma_start(xf32[:], xt[s])
                nc.sync.dma_start(xb32[:], xt[T - 1 - s])
                xf = sb.tile([128, KC, B], f32r, tag="xf")
                xb = sb.tile([128, KC, B], f32r, tag="xb")
                nc.vector.tensor_copy(xf[:], xf32.rearrange("p (c n) -> p c n", n=B))
                nc.vector.tensor_copy(xb[:], xb32.rearrange("p (c n) -> p c n", n=B))
                nc.tensor.matmul(bank_f[:], ones_t[:1, :], bf_t[:1, :],
                                 start=True, stop=False)
                nc.tensor.matmul(bank_b[:], ones_t[:1, :], bb_t[:1, :],
                                 start=True, stop=False)
                last = (s == 0)
                for k in range(KC):
                    st = (k == KC - 1) and last
                    nc.tensor.matmul(bank_f[:], xf[:, k, :], wihf_t[:, k, :],
                                     start=False, stop=st)
                    nc.tensor.matmul(bank_b[:], xb[:, k, :], wihb_t[:, k, :],
                                     start=False, stop=st)

            for s in range(min(D_LOOK + 1, T)):
                emit_xg(s)

            c_prev = c_zero
            hT_prev = None
            for t in range(T):
                bank_f, bank_b = banks.pop(t)
                if t > 0:
                    # all f matmuls first so sigmoid(f) overlaps the b matmuls
                    for k in range(KC):
                        nc.tensor.matmul(bank_f[:], hT_prev[:, k, 0:64],
                                         whhf_t[:, k, :], start=False,
                                         stop=(k == KC - 1))
                    for k in range(KC):
                        nc.tensor.matmul(bank_b[:], hT_prev[:, k, 64:128],
                                         whhb_t[:, k, :], start=False,
                                         stop=(k == KC - 1))
                S = sb.tile([128, 384], f32, tag="S")
                G = sb.tile([128, 128], f32, tag="G")
                ig = sb.tile([128, 128], f32, tag="ig")
                c_new = state.tile([128, 128], f32, tag="c")
                Tc = sb.tile([128, 128], f32, tag="Tc")
                h_t = sb.tile([128, 128], f32, tag="h")
                for lo, hi, bank in ((0, 64, bank_f), (64, 128, bank_b)):
                    nc.scalar.activation(S[lo:hi, :], bank[:, 0:384],
                                         mybir.ActivationFunctionType.Sigmoid)
                    nc.scalar.activation(G[lo:hi, :], bank[:, 384:512],
                                         mybir.ActivationFunctionType.Tanh)
                    nc.vector.tensor_mul(ig[lo:hi, :], S[lo:hi, 0:128], G[lo:hi, :])
                    nc.vector.tensor_mul(c_new[lo:hi, :], S[lo:hi, 128:256],
                                         c_prev[lo:hi, :])
                    nc.vector.tensor_add(c_new[lo:hi, :], c_new[lo:hi, :],
                                         ig[lo:hi, :])
                    nc.scalar.activation(Tc[lo:hi, :], c_new[lo:hi, :],
                                         mybir.ActivationFunctionType.Tanh)
                    nc.vector.tensor_mul(h_t[lo:hi, :], S[lo:hi, 256:384],
                                         Tc[lo:hi, :])
                c_prev = c_new

                if t < T - 1:
                    trp = ptr.tile([128, 128], f32, tag="tr")
                    nc.tensor.transpose(trp[:], h_t[:], ident[:])
                    hT_sb = sb.tile([128, 128], f32, tag="hTsb")
                    nc.vector.tensor_copy(hT_sb[:], trp[:])
                    b_in = dram.tile([128, 128], f32, tag="bin")
                    b_out = dram.tile([H, 128], f32, tag="bout")
                    nc.sync.dma_start(b_in[:], hT_sb[:])
                    nc.sync.dma_start(out_slab[t], h_t[:])
                    nc.gpsimd.collective_compute(
                        "AllGather", mybir.AluOpType.bypass,
                        replica_groups=[list(range(N_CORES))],
                        ins=[b_in[:].opt()], outs=[b_out[:].opt()],
                    )
                    if t + D_LOOK + 1 < T:
                        emit_xg(t + D_LOOK + 1)
                    warm = pwarm.tile([64, GL], f32, tag="warm")
                    for wk in range(22):
                        nc.tensor.matmul(warm[:], ones_t[:1, :], bf_t[:1, :],
                                         start=True, stop=True)
                    hT32 = sb.tile([128, KC, 128], f32, tag="hT32")
                    hT_new = state.tile([128, KC, 128], f32r, tag="hT")
                    for k in range(KC):
                        nc.sync.dma_start(hT32[:, k, :], b_out[k * 128:(k + 1) * 128, :])
                        nc.vector.tensor_copy(hT_new[:, k, :], hT32[:, k, :])
                    hT_prev = hT_new
                else:
                    nc.sync.dma_start(out_slab[t], h_t[:])
                    nc.sync.dma_start(hc_out[0], h_t[:])
                    nc.sync.dma_start(hc_out[1], c_new[:])
    nc.finalize()
    return nc


def _get_nc():
    if "nc" not in _CACHE:
        _CACHE["nc"] = _build()
    return _CACHE["nc"]


def _prep_inputs(x, w_ih_f, w_hh_f, b_ih_f, b_hh_f, w_ih_b, w_hh_b, b_ih_b, b_hh_b):
    x32 = np.asarray(x, np.float32)
    xt = np.ascontiguousarray(
        x32.transpose(0, 2, 1).reshape(T, KC, 128, B).transpose(0, 2, 1, 3)
        .reshape(T, 128, KC * B))
    ones = np.ones((1, B), np.float32)
    maps = []
    for c in range(N_CORES):
        rows = np.concatenate([np.arange(g * H + c * 128, g * H + (c + 1) * 128)
                               for g in (0, 1, 3, 2)])  # i, f, o, g
        maps.append({
            "xt": xt,
            "whh_f": np.ascontiguousarray(np.asarray(w_hh_f, np.float32)[rows].T),
            "whh_b": np.ascontiguousarray(np.asarray(w_hh_b, np.float32)[rows].T),
            "wih_f": np.ascontiguousarray(np.asarray(w_ih_f, np.float32)[rows].T),
            "wih_b": np.ascontiguousarray(np.asarray(w_ih_b, np.float32)[rows].T),
            "bias_f": (np.asarray(b_ih_f, np.float32)
                       + np.asarray(b_hh_f, np.float32))[rows][None, :],
            "bias_b": (np.asarray(b_ih_b, np.float32)
                       + np.asarray(b_hh_b, np.float32))[rows][None, :],
            "ones": ones,
        })
    return maps


def _assemble(results):
    out = np.zeros((T, B, 2 * H), np.float32)
    h = np.zeros((2, B, H), np.float32)
    c = np.zeros((2, B, H), np.float32)
    for ci, r in enumerate(results):
        slab = r["out_slab"]
        out[:, :, ci * 128:(ci + 1) * 128] = slab[:, 0:64, :]
        out[::-1, :, H + ci * 128:H + (ci + 1) * 128] = slab[:, 64:128, :]
        hc = r["hc_out"]
        h[0, :, ci * 128:(ci + 1) * 128] = hc[0, 0:64, :]
        h[1, :, ci * 128:(ci + 1) * 128] = hc[0, 64:128, :]
        c[0, :, ci * 128:(ci + 1) * 128] = hc[1, 0:64, :]
        c[1, :, ci * 128:(ci + 1) * 128] = hc[1, 64:128, :]
    return out, h, c


def kernel(x, w_ih_f, w_hh_f, b_ih_f, b_hh_f, w_ih_b, w_hh_b, b_ih_b, b_hh_b,
           _trace=False):
    from concourse.bass_utils import run_bass_kernel_spmd

    nc = _get_nc()
    maps = _prep_inputs(x, w_ih_f, w_hh_f, b_ih_f, b_hh_f,
                        w_ih_b, w_hh_b, b_ih_b, b_hh_b)
    res = run_bass_kernel_spmd(nc, maps, list(range(N_CORES)), trace=_trace)
    out, h, c = _assemble(res.results)
    if _trace:
        return (out, h, c), res.exec_time_ns
    return (out, h, c)


# revision 8
# speedup vs baseline: 1.2786x; 1.2786x over previous
"""Bidirectional LSTM layer (T=256, B=64, I=H=1024) on 8 Trainium2 NeuronCores.

Distribution: 8-way hidden split. Core c owns hidden units [128c, 128c+128)
of BOTH directions; its per-step gate slice is 512 cols per direction
(gate-col order [i|f|o|g] x 128, so sigmoid covers cols 0:384, tanh 384:512).

Per step t (one Bass program, fully unrolled, Tile-scheduled):
  - fp32r matmuls accumulate gates into per-direction PSUM banks:
    K=1 bias matmul + 8 input-projection (xg) matmuls (emitted D_LOOK steps
    ahead, so they execute inside the AllGather window) + 8 recurrent
    matmuls vs the allgathered h.T of step t-1.
  - sigmoid/tanh on ScalarE, c/h update on VectorE, on [128,*] tiles
    (forward batch in partitions 0:64, backward in 64:128).
  - h [128,128] is PE-transposed to h.T [hid, batch] and AllGathered
    (ncfw collective, DRAM bounce) into every core's next stationaries.
The backward direction is the same recurrence fed with time-reversed x
(pure host-side indexing); outputs are un-reversed at assembly.
"""
import numpy as np

T, B, I, H = 256, 64, 1024, 1024
N_CORES = 8
GL = 512          # local gate cols per core per direction
KC = 8            # K chunks (1024 / 128)
D_LOOK = 2        # xg lookahead steps (PSUM banks: 2*(D_LOOK+1) + 1 <= 8)

_CACHE = {}


def _build():
    import concourse.bacc as bacc
    import concourse.mybir as mybir
    import concourse.tile as tile
    from concourse.masks import make_identity

    dt = mybir.dt
    f32 = dt.float32
    f16 = dt.float16

    nc = bacc.Bacc(None)
    xt = nc.declare_dram_parameter("xt", [T, 128, KC * B], f32, isOutput=False)
    whh_f = nc.declare_dram_parameter("whh_f", [H, GL], f32, isOutput=False)
    whh_b = nc.declare_dram_parameter("whh_b", [H, GL], f32, isOutput=False)
    wih_f = nc.declare_dram_parameter("wih_f", [I, GL], f32, isOutput=False)
    wih_b = nc.declare_dram_parameter("wih_b", [I, GL], f32, isOutput=False)
    bias_f = nc.declare_dram_parameter("bias_f", [1, GL], f32, isOutput=False)
    bias_b = nc.declare_dram_parameter("bias_b", [1, GL], f32, isOutput=False)
    ones_in = nc.declare_dram_parameter("ones", [1, B], f32, isOutput=False)
    out_slab = nc.declare_dram_parameter("out_slab", [T, 128, 128], f32, isOutput=True)
    hc_out = nc.declare_dram_parameter("hc_out", [2, 128, 128], f32, isOutput=True)

    with tile.TileContext(nc) as tc:
        with (
            tc.tile_pool(name="const", bufs=1) as const,
            tc.tile_pool(name="sb", bufs=2) as sb,
            tc.tile_pool(name="state", bufs=2) as state,
            tc.tile_pool(name="pgates", bufs=2 * (D_LOOK + 1), space="PSUM") as pgates,
            tc.tile_pool(name="ptr", bufs=1, space="PSUM") as ptr,
            tc.tile_pool(name="pwarm", bufs=1, space="PSUM") as pwarm,
            tc.tile_pool(name="dram", bufs=2, space="DRAM") as dram,
        ):
            whhf_t = const.tile([128, KC, GL], f16, tag="whhf")
            whhb_t = const.tile([128, KC, GL], f16, tag="whhb")
            wihf_t = const.tile([128, KC, GL], f16, tag="wihf")
            wihb_t = const.tile([128, KC, GL], f16, tag="wihb")
            for tle, src in ((whhf_t, whh_f), (whhb_t, whh_b),
                             (wihf_t, wih_f), (wihb_t, wih_b)):
                nc.gpsimd.dma_start(tle[:], src.rearrange("(c p) n -> p c n", p=128))
            bf_t = const.tile([1, GL], f16, tag="bf")
            bb_t = const.tile([1, GL], f16, tag="bb")
            ones_t = const.tile([1, B], f16, tag="ones")
            nc.gpsimd.dma_start(bf_t[:], bias_f[:])
            nc.gpsimd.dma_start(bb_t[:], bias_b[:])
            nc.gpsimd.dma_start(ones_t[:], ones_in[:])
            ident = const.tile([128, 128], f32, tag="ident")
            make_identity(nc, ident)
            c_zero = const.tile([128, 128], f32, tag="czero")
            nc.vector.memset(c_zero[:], 0.0)

            banks = {}

            def emit_xg(s):
                bank_f = pgates.tile([64, GL], f32, tag="gates")
                bank_b = pgates.tile([64, GL], f32, tag="gates")
                banks[s] = (bank_f, bank_b)
                xf32 = sb.tile([128, KC * B], f32, tag="xf32")
                xb32 = sb.tile([128, KC * B], f32, tag="xb32")
                nc.sync.dma_start(xf32[:], xt[s])
                nc.sync.dma_start(xb32[:], xt[T - 1 - s])
                xf = sb.tile([128, KC, B], f16, tag="xf")
                xb = sb.tile([128, KC, B], f16, tag="xb")
                nc.vector.tensor_copy(xf[:], xf32.rearrange("p (c n) -> p c n", n=B))
                nc.vector.tensor_copy(xb[:], xb32.rearrange("p (c n) -> p c n", n=B))
                nc.tensor.matmul(bank_f[:], ones_t[:1, :], bf_t[:1, :],
                                 start=True, stop=False)
                nc.tensor.matmul(bank_b[:], ones_t[:1, :], bb_t[:1, :],
                                 start=True, stop=False)
                last = (s == 0)
                for k in range(KC):
                    st = (k == KC - 1) and last
                    nc.tensor.matmul(bank_f[:], xf[:, k, :], wihf_t[:, k, :],
                                     start=False, stop=st)
                    nc.tensor.matmul(bank_b[:], xb[:, k, :], wihb_t[:, k, :],
                                     start=False, stop=st)

            for s in range(min(D_LOOK + 1, T)):
                emit_xg(s)

            c_prev = c_zero
            hT_prev = None
            for t in range(T):
                bank_f, bank_b = banks.pop(t)
                if t > 0:
                    # all f matmuls first so sigmoid(f) overlaps the b matmuls
                    for k in range(KC):
                        nc.tensor.matmul(bank_f[:], hT_prev[:, k, 0:64],
                                         whhf_t[:, k, :], start=False,
                                         stop=(k == KC - 1))
                    for k in range(KC):
                        nc.tensor.matmul(bank_b[:], hT_prev[:, k, 64:128],
                                         whhb_t[:, k, :], start=False,
                                         stop=(k == KC - 1))
                S = sb.tile([128, 384], f32, tag="S")
                G = sb.tile([128, 128], f32, tag="G")
                ig = sb.tile([128, 128], f32, tag="ig")
                c_new = state.tile([128, 128], f32, tag="c")
                Tc = sb.tile([128, 128], f32, tag="Tc")
                h_t = sb.tile([128, 128], f32, tag="h")
                for lo, hi, bank in ((0, 64, bank_f), (64, 128, bank_b)):
                    nc.scalar.activation(S[lo:hi, :], bank[:, 0:384],
                                         mybir.ActivationFunctionType.Sigmoid)
                    nc.scalar.activation(G[lo:hi, :], bank[:, 384:512],
                                         mybir.ActivationFunctionType.Tanh)
                    nc.vector.tensor_mul(ig[lo:hi, :], S[lo:hi, 0:128], G[lo:hi, :])
                    nc.vector.tensor_mul(c_new[lo:hi, :], S[lo:hi, 128:256],
                                         c_prev[lo:hi, :])
                    nc.vector.tensor_add(c_new[lo:hi, :], c_new[lo:hi, :],
                                         ig[lo:hi, :])
                    nc.scalar.activation(Tc[lo:hi, :], c_new[lo:hi, :],
                                         mybir.ActivationFunctionType.Tanh)
                    nc.vector.tensor_mul(h_t[lo:hi, :], S[lo:hi, 256:384],
                                         Tc[lo:hi, :])
                nc.sync.dma_start(out_slab[t], h_t[:])
                c_prev = c_new

                if t < T - 1:
                    trp = ptr.tile([128, 128], f32, tag="tr")
                    nc.tensor.transpose(trp[:], h_t[:], ident[:])
                    hT_sb = sb.tile([128, 128], f16, tag="hTsb")
                    nc.vector.tensor_copy(hT_sb[:], trp[:])
                    b_in = dram.tile([128, 128], f16, tag="bin")
                    b_out = dram.tile([H, 128], f16, tag="bout")
                    nc.sync.dma_start(b_in[:], hT_sb[:])
                    nc.gpsimd.collective_compute(
                        "AllGather", mybir.AluOpType.bypass,
                        replica_groups=[list(range(N_CORES))],
                        ins=[b_in[:].opt()], outs=[b_out[:].opt()],
                    )
                    if t + D_LOOK + 1 < T:
                        emit_xg(t + D_LOOK + 1)
                    warm = pwarm.tile([64, GL], f32, tag="warm")
                    for wk in range(6):
                        nc.tensor.matmul(warm[:], ones_t[:1, :], bf_t[:1, :],
                                         start=True, stop=True)
                    hT_new = state.tile([128, KC, 128], f16, tag="hT")
                    for k in range(KC):
                        nc.sync.dma_start(hT_new[:, k, :],
                                          b_out[k * 128:(k + 1) * 128, :])
                    hT_prev = hT_new
                else:
                    nc.sync.dma_start(hc_out[0], h_t[:])
                    nc.sync.dma_start(hc_out[1], c_new[:])
    nc.finalize()
    return nc


def _get_nc():
    if "nc" not in _CACHE:
        _CACHE["nc"] = _build()
    return _CACHE["nc"]


def _prep_inputs(x, w_ih_f, w_hh_f, b_ih_f, b_hh_f, w_ih_b, w_hh_b, b_ih_b, b_hh_b):
    x32 = np.asarray(x, np.float32)
    xt = np.ascontiguousarray(
        x32.transpose(0, 2, 1).reshape(T, KC, 128, B).transpose(0, 2, 1, 3)
        .reshape(T, 128, KC * B))
    ones = np.ones((1, B), np.float32)
    maps = []
    for c in range(N_CORES):
        rows = np.concatenate([np.arange(g * H + c * 128, g * H + (c + 1) * 128)
                               for g in (0, 1, 3, 2)])  # i, f, o, g
        maps.append({
            "xt": xt,
            "whh_f": np.ascontiguousarray(np.asarray(w_hh_f, np.float32)[rows].T),
            "whh_b": np.ascontiguousarray(np.asarray(w_hh_b, np.float32)[rows].T),
            "wih_f": np.ascontiguousarray(np.asarray(w_ih_f, np.float32)[rows].T),
            "wih_b": np.ascontiguousarray(np.asarray(w_ih_b, np.float32)[rows].T),
            "bias_f": (np.asarray(b_ih_f, np.float32)
                       + np.asarray(b_hh_f, np.float32))[rows][None, :],
            "bias_b": (np.asarray(b_ih_b, np.float32)
                       + np.asarray(b_hh_b, np.float32))[rows][None, :],
            "ones": ones,
        })
    return maps


def _assemble(results):
    out = np.zeros((T, B, 2 * H), np.float32)
    h = np.zeros((2, B, H), np.float32)
    c = np.zeros((2, B, H), np.float32)
    for ci, r in enumerate(results):
        slab = r["out_slab"]
        out[:, :, ci * 128:(ci + 1) * 128] = slab[:, 0:64, :]
        out[::-1, :, H + ci * 128:H + (ci + 1) * 128] = slab[:, 64:128, :]
        hc = r["hc_out"]
        h[0, :, ci * 128:(ci + 1) * 128] = hc[0, 0:64, :]
        h[1, :, ci * 128:(ci + 1) * 128] = hc[0, 64:128, :]
        c[0, :, ci * 128:(ci + 1) * 128] = hc[1, 0:64, :]
        c[1, :, ci * 128:(ci + 1) * 128] = hc[1, 64:128, :]
    return out, h, c


def kernel(x, w_ih_f, w_hh_f, b_ih_f, b_hh_f, w_ih_b, w_hh_b, b_ih_b, b_hh_b,
           _trace=False):
    from concourse.bass_utils import run_bass_kernel_spmd

    nc = _get_nc()
    maps = _prep_inputs(x, w_ih_f, w_hh_f, b_ih_f, b_hh_f,
                        w_ih_b, w_hh_b, b_ih_b, b_hh_b)
    res = run_bass_kernel_spmd(nc, maps, list(range(N_CORES)), trace=_trace)
    out, h, c = _assemble(res.results)
    if _trace:
        return (out, h, c), res.exec_time_ns
    return (out, h, c)
